# revision 74
# baseline (speedup 1.0000x reference)
"""Tensor-parallel attention kernel for Trainium2 (8 NeuronCores), v2.

Problem: S=2048, B=2, Dm=2048, H=16, Dh=128 attention layer with per-head
RMSNorm (q,k) + RoPE + SDPA + output projection.

Sharding: tensor-parallel over heads. Core c owns heads {2c, 2c+1}:
Wq/Wk/Wv sharded by output rows (256 rows per core), Wo by columns; each
core computes a full-shape partial of the output projection and the host
sums the 8 partials.

Design (measured ~402us vs 474.8us f32r baseline; rel err ~7.7e-3; the two
head-blocks of each query block run pair-interleaved so each head's exp
latency hides behind the other head's score/AV matmuls):
 - all matmul operands in bf16 (same PE rate as f32r at free>=256, half
   the DMA bytes); PSUM accumulation stays fp32. Output partials bf16,
   summed on host.
 - phase A: per 128-token chunk, 512-wide qk + 256-wide v matmuls; RMS
   stats via ACT Square+accum_out; RoPE on a bf16 copy with the norm
   weights folded into bf16 cos/sin factor tiles; q^T/k^T via PE
   transposes (bf16, 1c/row) deferred 3 chunks behind the matmul stream
   so the cos/sin build never stalls the in-order PE queue.
 - phase B: scores into 2-bank PSUM tiles, 1024-wide ACT exp to bf16;
   full-tile [128,1024] DVE presums (contiguous bf16 hits the 2x mode);
   denominator pre-broadcast via an all-ones [128,128] stationary matmul
   then reciprocal_approx_fast on [128,512] (no gpsimd broadcast).
 - output projection of block j interleaved 1-unit-per-pair into block
   j+1's score/AV stream as PE filler; den tile shares a 2-buffer PSUM
   pool with the projection tiles (allocated just before use); PSUM->SBUF
   copies split 1:3 over ACT:DVE.
 - PSUM: A = ppqk2+ppv2+pptr2; B = ppsc 2x2banks + ppav2 + shared 2.
Pitfalls hit on the way: dma_start(transpose=True) costs ~1.25us of the
issuing engine per call AND lost a race (readers before DMA landed);
gpsimd ops (~1.1us per 512-elem add, no PSUM access) poison serial
chains; AP partition_broadcast isn't readable by DVE.
"""
import sys

for _p in ("/opt/trn_rl_repo", "/root/.axon_site/_ro/trn_rl_repo"):
    if _p not in sys.path:
        sys.path.append(_p)

import math
import numpy as np
import ml_dtypes

import concourse.bass as bass
import concourse.tile as tile
from concourse import bacc, mybir
from concourse import bass_utils
from concourse.masks import make_identity

F32 = mybir.dt.float32
BF16 = mybir.dt.bfloat16
AF = mybir.ActivationFunctionType
MUL = mybir.AluOpType.mult
ADD = mybir.AluOpType.add
SUB = mybir.AluOpType.subtract

S, B, DM, H, DH = 2048, 2, 2048, 16, 128
NC = 8                 # cores
HC = H // NC           # heads per core = 2
JC = HC * DH           # per-core inner dim = 256
T = S * B              # tokens = 4096
KO = DM // 128         # contraction chunks = 16
TCH = T // 128         # token chunks = 32
SCH = S // 128         # per-batch chunks = 16
EPS = 1e-6
BF = ml_dtypes.bfloat16

_CACHE = {}


def _build(g_ones):
    nc = bacc.Bacc(trn_type="TRN2", target_bir_lowering=False, debug=False,
                   num_devices=NC)

    xT_d = nc.dram_tensor("xT", [TCH, 128, KO, 128], BF16,
                          kind="ExternalInput").ap()
    wqkv_d = nc.dram_tensor("wqkv", [DM, 3 * JC], BF16,
                            kind="ExternalInput").ap()
    wo_d = nc.dram_tensor("woT", [JC, DM], BF16, kind="ExternalInput").ap()
    rope_d = nc.dram_tensor("rope", [S, DH // 2], F32,
                            kind="ExternalInput").ap()
    gq_d = nc.dram_tensor("gq", [1, DH], F32, kind="ExternalInput").ap()
    gk_d = nc.dram_tensor("gk", [1, DH], F32, kind="ExternalInput").ap()
    out_d = nc.dram_tensor("out", [T, DM], BF16, kind="ExternalOutput").ap()

    with tile.TileContext(nc) as tc:
        with tc.tile_pool(name="persist", bufs=1) as persist:
            qT = persist.tile([128, HC, T], BF16)    # d on partitions
            kT = persist.tile([128, HC, T], BF16)
            v_sb = persist.tile([128, TCH, JC], BF16)  # tokens on partitions
            wo = persist.tile([128, HC, DM], BF16)

            # ---------------- Phase A: projections + norm + rope ----------
            with tc.tile_pool(name="pha", bufs=1) as pha, \
                 tc.tile_pool(name="wka", bufs=3) as wka, \
                 tc.tile_pool(name="xin", bufs=4) as xin, \
                 tc.tile_pool(name="ppqk", bufs=2, space="PSUM") as ppqk, \
                 tc.tile_pool(name="ppv", bufs=2, space="PSUM") as ppv, \
                 tc.tile_pool(name="pptr", bufs=2, space="PSUM") as pptr:

                # prefetch: first x chunk before the weights so the PE can
                # start as soon as wqkv[0] lands.
                def load_xc(tcch):
                    t = xin.tile([128, KO, 128], BF16, tag="xc")
                    eng = nc.scalar if tcch % 2 == 0 else nc.sync
                    eng.dma_start(t[:], xT_d[tcch])
                    return t

                # dummy Sin on a ready tile: pulls the ACT sin table set in
                # at t=0, concurrent with the rope DMA, so the real Sins
                # (head of the strict-FIFO ACT queue) don't pay the load.
                dmy = pha.tile([128, 1], F32)
                nc.vector.memset(dmy[:], 0.0)
                dmy2 = pha.tile([128, 1], F32)
                nc.scalar.activation(dmy2[:], dmy[:], AF.Sin, bias=0.0)

                # rope angles first (small, feeds the Sin -> rope chain that
                # gates the first chunks' SBUF recycling), on the queue the
                # first x chunk does NOT use.
                rope_sb = pha.tile([128, SCH, 64], F32)
                nc.sync.dma_start(
                    rope_sb[:], rope_d.rearrange("(rc p) d -> p rc d", p=128))

                xc_next = load_xc(0)
                xc_q = [load_xc(1), load_xc(2)]

                wqkv_src = wqkv_d.rearrange("(ko ki) n -> ki ko n", ki=128)
                wqkv = []
                for ko in range(KO):
                    wk_t = pha.tile([128, 3 * JC], BF16, tag=f"wqkv{ko}")
                    eng = nc.sync if ko % 2 == 0 else nc.scalar
                    eng.dma_start(wk_t[:], wqkv_src[:, ko, :])
                    wqkv.append(wk_t)

                if not g_ones:
                    g_sb = pha.tile([1, 2, DH], F32)
                    nc.scalar.dma_start(g_sb[:, 0, :], gq_d[:])
                    nc.scalar.dma_start(g_sb[:, 1, :], gk_d[:])

                epsb = pha.tile([128, 1], F32)
                nc.vector.memset(epsb[:], float(DH * EPS))
                ident = pha.tile([128, 128], BF16)
                make_identity(nc, ident[:])

                # ACT Sin needs args in [-pi, pi]. Single fold (valid for
                # |x + shift| < 3pi; angles are O(1) randn):
                PI, TWOPI = float(np.pi), float(2 * np.pi)

                def wrapped_sin(dst, shift):
                    xs = pha.tile([128, SCH, 64], F32, tag="w_xs")
                    if shift:
                        nc.vector.tensor_scalar_add(xs[:], rope_sb[:], shift)
                    else:
                        nc.vector.tensor_copy(xs[:], rope_sb[:])
                    hi = pha.tile([128, SCH, 64], F32, tag="w_m")
                    nc.vector.tensor_scalar(hi[:], xs[:], PI, TWOPI,
                                            mybir.AluOpType.is_gt, MUL)
                    nc.vector.tensor_tensor(xs[:], xs[:], hi[:], SUB)
                    lo = pha.tile([128, SCH, 64], F32, tag="w_m")
                    nc.vector.tensor_scalar(lo[:], xs[:], -PI, TWOPI,
                                            mybir.AluOpType.is_lt, MUL)
                    nc.vector.tensor_tensor(xs[:], xs[:], lo[:], ADD)
                    nc.scalar.activation(dst[:], xs[:], AF.Sin, bias=0.0)

                if g_ones:
                    # RMSNorm weights are 1: skip the factor fold entirely;
                    # rope reads the bf16 cos/sin tiles directly and the
                    # first chunk's rope only waits on the Sin chain.
                    cos_sb = pha.tile([128, SCH, 64], BF16)
                    sin_sb = pha.tile([128, SCH, 64], BF16)
                    wrapped_sin(sin_sb, 0.0)
                    wrapped_sin(cos_sb, float(np.pi / 2))
                else:
                    cos_f = pha.tile([128, SCH, 64], F32)
                    sin_f = pha.tile([128, SCH, 64], F32)
                    wrapped_sin(sin_f, 0.0)
                    wrapped_sin(cos_f, float(np.pi / 2))

                    # fold the RMSNorm weights into the rotation factors:
                    # o1 = x1*(g1*c) - x2*(g2*s); o2 = x1*(g1*s) + x2*(g2*c)
                    C1 = pha.tile([128, SCH, 2, 64], BF16)
                    S1 = pha.tile([128, SCH, 2, 64], BF16)
                    C2 = pha.tile([128, SCH, 2, 64], BF16)
                    S2 = pha.tile([128, SCH, 2, 64], BF16)
                    gb = pha.tile([128, 2, DH], F32)
                    nc.gpsimd.partition_broadcast(gb[:], g_sb[:])
                    for t in range(2):
                        g1 = gb[:, t, None, 0:64].broadcast_to(
                            (128, SCH, 64))
                        g2 = gb[:, t, None, 64:128].broadcast_to(
                            (128, SCH, 64))
                        nc.vector.tensor_tensor(C1[:, :, t, :], cos_f[:],
                                                g1, MUL)
                        nc.vector.tensor_tensor(S1[:, :, t, :], sin_f[:],
                                                g1, MUL)
                        nc.vector.tensor_tensor(C2[:, :, t, :], cos_f[:],
                                                g2, MUL)
                        nc.vector.tensor_tensor(S2[:, :, t, :], sin_f[:],
                                                g2, MUL)

                # transposes of chunk i are emitted after chunk i+2's
                # matmuls: the in-order PE queue then has matmul runway
                # while chunk 0/1's rope chain (cos/sin/cfac build) settles.
                pending_tr = []

                for tcch in range(TCH):
                    sc = tcch % SCH  # chunk index within batch (rope rows)
                    xc = xc_next
                    if xc_q:
                        xc_next = xc_q.pop(0)
                    if tcch + 3 < TCH:
                        xc_q.append(load_xc(tcch + 3))
                    if tcch == 8:
                        # wo is first needed by the output projection of
                        # (b0, sj0) — load it after the startup rush
                        wo_src = wo_d.rearrange("(h ki) n -> ki h n", ki=128)
                        for h in range(HC):
                            nc.sync.dma_start(wo[:, h, :], wo_src[:, h, :])

                    ps_qk = ppqk.tile([128, 2 * JC], F32, tag="psqk")
                    ps_v = ppv.tile([128, JC], F32, tag="psv")
                    for ko in range(KO):
                        nc.tensor.matmul(ps_qk[:], xc[:, ko, :],
                                         wqkv[ko][:, 0:2 * JC],
                                         start=(ko == 0), stop=(ko == KO - 1))
                        nc.tensor.matmul(ps_v[:], xc[:, ko, :],
                                         wqkv[ko][:, 2 * JC:3 * JC],
                                         start=(ko == 0), stop=(ko == KO - 1))
                    if len(pending_tr) > 2:
                        pending_tr.pop(0)()
                        # drain the backlog over the last chunks so the
                        # end-of-A flush only waits on the final rope tail
                        if tcch >= TCH - 4 and pending_tr:
                            pending_tr.pop(0)()
                    nc.scalar.copy(v_sb[:, tcch, :], ps_v[:])

                    # rms stats over each head's 128 dims (q:2 heads, k:2)
                    ssq = wka.tile([128, 4], F32, tag="ssq")
                    sqd = wka.tile([128, DH], BF16, tag="sqd")
                    for th in range(4):
                        nc.scalar.activation(
                            sqd[:], ps_qk[:, th * DH:(th + 1) * DH],
                            AF.Square, accum_out=ssq[:, th:th + 1])
                    # q side folds 1/sqrt(DH): 1/sqrt(ssq + DH*eps)
                    rr = wka.tile([128, 4], F32, tag="rr")
                    nc.scalar.activation(rr[:], ssq[:], AF.Sqrt, bias=epsb[:])
                    rr2 = wka.tile([128, 4], F32, tag="rr2")
                    nc.vector.reciprocal(rr2[:], rr[:])
                    # k side: 1/sqrt(ssq/DH+eps) = sqrt(DH)/sqrt(ssq+DH*eps)
                    nc.vector.tensor_scalar_mul(rr2[:, 2:4], rr2[:, 2:4],
                                                float(math.sqrt(DH)))
                    rr2b = wka.tile([128, 4], BF16, tag="rr2b")
                    nc.vector.tensor_copy(rr2b[:], rr2[:])

                    qkc = wka.tile([128, 2, HC, 2, 64], BF16, tag="qkc")
                    nc.vector.tensor_copy(
                        qkc[:].rearrange("p t h f d -> p (t h f d)"),
                        ps_qk[:])

                    if g_ones:
                        c1 = (cos_sb[:, sc, None, None, :]
                              .broadcast_to((128, 2, HC, 64)))
                        s1 = (sin_sb[:, sc, None, None, :]
                              .broadcast_to((128, 2, HC, 64)))
                        c2, s2 = c1, s1
                    else:
                        c1 = (C1[:, sc, :, None, :]
                              .broadcast_to((128, 2, HC, 64)))
                        s1 = (S1[:, sc, :, None, :]
                              .broadcast_to((128, 2, HC, 64)))
                        c2 = (C2[:, sc, :, None, :]
                              .broadcast_to((128, 2, HC, 64)))
                        s2 = (S2[:, sc, :, None, :]
                              .broadcast_to((128, 2, HC, 64)))
                    x1 = qkc[:, :, :, 0, :]
                    x2 = qkc[:, :, :, 1, :]
                    t1 = wka.tile([128, 2, HC, 64], BF16, tag="t1")
                    t2 = wka.tile([128, 2, HC, 64], BF16, tag="t2")
                    tr = wka.tile([128, 2, HC, 2, 64], BF16, tag="tr")
                    nc.vector.tensor_tensor(t1[:], x1, c1, MUL)
                    nc.vector.tensor_tensor(t2[:], x2, s2, MUL)
                    nc.vector.tensor_tensor(tr[:, :, :, 0, :], t1[:], t2[:],
                                            SUB)
                    nc.vector.tensor_tensor(t1[:], x1, s1, MUL)
                    nc.vector.tensor_tensor(t2[:], x2, c2, MUL)
                    nc.vector.tensor_tensor(tr[:, :, :, 1, :], t1[:], t2[:],
                                            ADD)
                    trr = wka.tile([128, 2, HC, DH], BF16, tag="trr")
                    nc.vector.tensor_tensor(
                        trr[:].rearrange("p t h d -> p (t h) d"),
                        tr[:].rearrange("p t h f d -> p (t h) (f d)"),
                        rr2b[:, :, None].broadcast_to((128, 2 * HC, DH)),
                        MUL)
                    def make_tr(trr_t, tcch_):
                        def emit_tr():
                            for t in range(2):
                                dstT = qT if t == 0 else kT
                                for h in range(HC):
                                    ps_tr = pptr.tile([128, 128], BF16,
                                                      tag="pstr")
                                    nc.tensor.transpose(ps_tr[:],
                                                        trr_t[:, t, h, :],
                                                        ident[:])
                                    dsl = dstT[:, h,
                                               tcch_ * 128:(tcch_ + 1) * 128]
                                    if (t * HC + h) % 2 == 0:
                                        nc.scalar.copy(dsl, ps_tr[:])
                                    else:
                                        nc.vector.tensor_copy(dsl, ps_tr[:])
                        return emit_tr

                    pending_tr.append(make_tr(trr, tcch))
                while pending_tr:
                    pending_tr.pop(0)()
                # preload the ACT exp table set during the A->B drain so
                # phase B's first exp doesn't pay the ~1.3us table load
                nc.scalar.activation(dmy2[:], dmy[:], AF.Exp)

            # ---------------- Phase B/C: SDPA + output projection ---------
            with tc.tile_pool(name="phb", bufs=1) as phb, \
                 tc.tile_pool(name="etp", bufs=8) as etp, \
                 tc.tile_pool(name="esp", bufs=2) as esp, \
                 tc.tile_pool(name="otp", bufs=2) as otp, \
                 tc.tile_pool(name="wkb", bufs=2) as wkb, \
                 tc.tile_pool(name="osbp", bufs=8) as osbp, \
                 tc.tile_pool(name="ppsc", bufs=2, space="PSUM") as ppsc, \
                 tc.tile_pool(name="ppav", bufs=2, space="PSUM") as ppav, \
                 tc.tile_pool(name="ppx", bufs=2, space="PSUM") as ppx:

                ones128 = phb.tile([128, 128], BF16)
                nc.vector.memset(ones128[:], 1.0)

                # Output-projection work of block j is interleaved into
                # block j+1's score/AV stream as PE filler.
                pending_c = []
                copy_rr = [0]

                def emit_c(n):
                    for _ in range(min(n, len(pending_c))):
                        pending_c.pop(0)()

                def make_c_unit(outT_t, b, sj, mi, oj, osb_t):
                    def unit():
                        ps_o = ppx.tile([128, 512], F32, tag="x")
                        for h in range(HC):
                            nc.tensor.matmul(
                                ps_o[:],
                                outT_t[:, h, mi * 128:(mi + 1) * 128],
                                wo[:, h, oj * 512:(oj + 1) * 512],
                                start=(h == 0), stop=(h == HC - 1))
                        osl = osb_t[:, oj * 512:(oj + 1) * 512]
                        if copy_rr[0] % 4 == 0:
                            nc.scalar.copy(osl, ps_o[:])
                        else:
                            nc.vector.tensor_copy(osl, ps_o[:])
                        copy_rr[0] += 1
                        if oj == 3:
                            m0 = b * S + sj * 512 + mi * 128
                            deng = nc.sync if mi % 2 == 0 else nc.scalar
                            deng.dma_start(out_d[m0:m0 + 128, :], osb_t[:])
                    return unit

                for b in range(B):
                    for sj in range(4):  # 512-query blocks within batch b
                        s0 = b * S + sj * 512
                        outT = otp.tile([128, HC, 512], BF16, tag="outT")
                        # the two head-blocks run interleaved at pair
                        # granularity: each head's exp latency hides behind
                        # the other head's score/AV matmuls. ppav/esp hand
                        # out one buffer per head from the same 2-buf tag.
                        ps_av0 = ppav.tile([128, 512], F32, tag="psav")
                        ps_av1 = ppav.tile([128, 512], F32, tag="psav")
                        es0 = esp.tile([128, 2, 512], BF16, tag="es")
                        es1 = esp.tile([128, 2, 512], BF16, tag="es")
                        ps_av = [ps_av0, ps_av1]
                        es = [es0, es1]
                        eprev = [None, None]

                        def pe_av(h, eT_t, pp):
                            for i in range(2):
                                ti = 2 * pp + i
                                nc.tensor.matmul(
                                    ps_av[h][:],
                                    v_sb[:, b * SCH + ti,
                                         h * DH:(h + 1) * DH],
                                    eT_t[:, i, :],
                                    start=(ti == 0),
                                    stop=(ti == SCH - 1))

                        for p in range(8):
                            for h in range(HC):
                                ps_sc = ppsc.tile([128, 2, 512], F32,
                                                  tag="pssc")
                                for i in range(2):
                                    ti = 2 * p + i
                                    nc.tensor.matmul(
                                        ps_sc[:, i, :],
                                        kT[:, h, b * S + ti * 128:
                                           b * S + (ti + 1) * 128],
                                        qT[:, h, s0:s0 + 512],
                                        start=True, stop=True)
                                eT = etp.tile([128, 2, 512], BF16, tag="eT")
                                nc.scalar.activation(
                                    eT[:].rearrange("p a b -> p (a b)"),
                                    ps_sc[:].rearrange("p a b -> p (a b)"),
                                    AF.Exp)
                                emit_c(1)
                                # full-tile presum: es[h] accumulates the 8
                                # eT pairs as [128, 1024] adds (7 total)
                                if p == 1:
                                    nc.vector.tensor_tensor(
                                        es[h][:], eprev[h][:], eT[:], ADD)
                                elif p > 1:
                                    nc.vector.tensor_tensor(
                                        es[h][:], es[h][:], eT[:], ADD)
                                if eprev[h] is not None:
                                    pe_av(h, eprev[h], p - 1)
                                eprev[h] = eT
                        for h in range(HC):
                            pe_av(h, eprev[h], 7)

                        for h in range(HC):
                            # denominator, pre-broadcast across partitions:
                            # ps_db[i, q] = sum_k es[k, q] for every row i
                            # (allocated here, between C-unit allocations of
                            # the same pool tag, so the 2-buffer rotation
                            # never makes a C matmul wait on the recip)
                            ps_db = ppx.tile([128, 512], F32, tag="x")
                            nc.tensor.matmul(ps_db[:], ones128[:],
                                             es[h][:, 0, :],
                                             start=True, stop=False)
                            nc.tensor.matmul(ps_db[:], ones128[:],
                                             es[h][:, 1, :],
                                             start=False, stop=True)
                            recb = wkb.tile([128, 512], F32, tag="recb")
                            nc.vector.reciprocal_approx_fast(recb[:],
                                                             ps_db[:])
                            nc.vector.tensor_tensor(
                                outT[:, h, :], ps_av[h][:], recb[:], MUL)

                        for mi in range(4):
                            osb_t = osbp.tile([128, DM], BF16, tag="osb")
                            for oj in range(4):
                                pending_c.append(
                                    make_c_unit(outT, b, sj, mi, oj, osb_t))
                emit_c(len(pending_c))

    nc.compile()
    return nc


def _get_program(g_ones):
    key = ("prog", g_ones)
    if key not in _CACHE:
        _CACHE[key] = _build(g_ones)
    return _CACHE[key]


def _prep_inputs(x, rope_emb, Wq, Wk, Wv, Wo, gq, gk):
    x = np.asarray(x, dtype=np.float32)
    # b-major tokens: row r = b*S + s
    xbm = x.transpose(1, 0, 2).reshape(T, DM)
    xT = np.ascontiguousarray(
        xbm.reshape(TCH, 128, KO, 128).transpose(0, 3, 2, 1)).astype(BF)
    rope = np.ascontiguousarray(
        np.asarray(rope_emb, dtype=np.float32).reshape(S, DH)[:, :DH // 2])
    gq2 = np.asarray(gq, dtype=np.float32).reshape(1, DH)
    gk2 = np.asarray(gk, dtype=np.float32).reshape(1, DH)
    Wq = np.asarray(Wq, dtype=np.float32)
    Wk = np.asarray(Wk, dtype=np.float32)
    Wv = np.asarray(Wv, dtype=np.float32)
    Wo = np.asarray(Wo, dtype=np.float32)
    in_maps = []
    for c in range(NC):
        r0, r1 = c * JC, (c + 1) * JC
        wqkv = np.ascontiguousarray(np.concatenate(
            [Wq[r0:r1].T, Wk[r0:r1].T, Wv[r0:r1].T], axis=1)).astype(BF)
        woT = np.ascontiguousarray(Wo[:, r0:r1].T).astype(BF)
        in_maps.append({"xT": xT, "wqkv": wqkv, "woT": woT, "rope": rope,
                        "gq": gq2, "gk": gk2})
    g_ones = bool(np.all(gq2 == 1.0) and np.all(gk2 == 1.0))
    return in_maps, g_ones


def _gather(results):
    acc = results[0]["out"].astype(np.float32)
    for r in results[1:]:
        acc += r["out"].astype(np.float32)
    out = acc.reshape(B, S, DM).transpose(1, 0, 2)
    return np.ascontiguousarray(out)


def kernel(x, rope_emb, Wq, Wk, Wv, Wo, gq, gk):
    in_maps, g_ones = _prep_inputs(x, rope_emb, Wq, Wk, Wv, Wo, gq, gk)
    nc = _get_program(g_ones)
    res = bass_utils.run_bass_kernel_spmd(nc, in_maps,
                                          core_ids=list(range(NC)))
    return _gather(res.results)


def kernel_profiled(x, rope_emb, Wq, Wk, Wv, Wo, gq, gk):
    """Like kernel() but with NTFF tracing; returns (out, exec_time_ns)."""
    _install_ntff()
    in_maps, g_ones = _prep_inputs(x, rope_emb, Wq, Wk, Wv, Wo, gq, gk)
    nc = _get_program(g_ones)
    res = bass_utils.run_bass_kernel_spmd(nc, in_maps,
                                          core_ids=list(range(NC)),
                                          trace=True)
    return _gather(res.results), res.exec_time_ns


def _install_ntff():
    import contextlib
    import ctypes
    import types

    if "antenv.axon_hooks" in sys.modules:
        return
    so_path = "/opt/axon/libaxon_pjrt.so"
    try:
        lib = ctypes.CDLL(so_path)
    except OSError:
        return
    if not hasattr(lib, "axon_start_nrt_profile"):
        return
    lib.axon_start_nrt_profile.argtypes = [ctypes.POINTER(ctypes.c_int64),
                                           ctypes.c_size_t]
    lib.axon_start_nrt_profile.restype = ctypes.c_int64
    lib.axon_stop_nrt_profile.argtypes = [ctypes.c_char_p]
    lib.axon_stop_nrt_profile.restype = ctypes.c_int64

    @contextlib.contextmanager
    def hook(output_dir, device_ids):
        import jax
        jax.devices()
        if device_ids:
            ids = (ctypes.c_int64 * len(device_ids))(*device_ids)
            rc = lib.axon_start_nrt_profile(ids, len(device_ids))
        else:
            rc = lib.axon_start_nrt_profile(None, 0)
        if rc != 0:
            raise RuntimeError(f"axon_start_nrt_profile rc={rc}")
        try:
            yield
        finally:
            n = lib.axon_stop_nrt_profile(str(output_dir).encode())
            print(f"ntff profile: {n} file(s) -> {output_dir}", file=sys.stderr)

    mod = types.ModuleType("antenv.axon_hooks")
    _state = {"h": hook}
    mod.get_axon_ntff_profile_hook = lambda: _state["h"]
    mod.set_axon_ntff_profile_hook = lambda h: _state.__setitem__("h", h)
    sys.modules["antenv.axon_hooks"] = mod


# revision 77
# speedup vs baseline: 1.0037x; 1.0037x over previous
"""Tensor-parallel attention kernel for Trainium2 (8 NeuronCores), v2.

Problem: S=2048, B=2, Dm=2048, H=16, Dh=128 attention layer with per-head
RMSNorm (q,k) + RoPE + SDPA + output projection.

Sharding: tensor-parallel over heads. Core c owns heads {2c, 2c+1}:
Wq/Wk/Wv sharded by output rows (256 rows per core), Wo by columns; each
core computes a full-shape partial of the output projection and the host
sums the 8 partials.

Design (measured ~402us vs 474.8us f32r baseline; rel err ~7.7e-3; the two
head-blocks of each query block run pair-interleaved so each head's exp
latency hides behind the other head's score/AV matmuls):
 - all matmul operands in bf16 (same PE rate as f32r at free>=256, half
   the DMA bytes); PSUM accumulation stays fp32. Output partials bf16,
   summed on host.
 - phase A: per 128-token chunk, 512-wide qk + 256-wide v matmuls; RMS
   stats via ACT Square+accum_out; RoPE on a bf16 copy with the norm
   weights folded into bf16 cos/sin factor tiles; q^T/k^T via PE
   transposes (bf16, 1c/row) deferred 3 chunks behind the matmul stream
   so the cos/sin build never stalls the in-order PE queue.
 - phase B: scores into 2-bank PSUM tiles, 1024-wide ACT exp to bf16;
   full-tile [128,1024] DVE presums (contiguous bf16 hits the 2x mode);
   denominator pre-broadcast via an all-ones [128,128] stationary matmul
   then reciprocal_approx_fast on [128,512] (no gpsimd broadcast).
 - output projection of block j interleaved 1-unit-per-pair into block
   j+1's score/AV stream as PE filler; den tile shares a 2-buffer PSUM
   pool with the projection tiles (allocated just before use); PSUM->SBUF
   copies split 1:3 over ACT:DVE.
 - PSUM: A = ppqk2+ppv2+pptr2; B = ppsc 2x2banks + ppav2 + shared 2.
Pitfalls hit on the way: dma_start(transpose=True) costs ~1.25us of the
issuing engine per call AND lost a race (readers before DMA landed);
gpsimd ops (~1.1us per 512-elem add, no PSUM access) poison serial
chains; AP partition_broadcast isn't readable by DVE.
"""
import sys

for _p in ("/opt/trn_rl_repo", "/root/.axon_site/_ro/trn_rl_repo"):
    if _p not in sys.path:
        sys.path.append(_p)

import math
import numpy as np
import ml_dtypes

import concourse.bass as bass
import concourse.tile as tile
from concourse import bacc, mybir
from concourse import bass_utils
from concourse.masks import make_identity

F32 = mybir.dt.float32
BF16 = mybir.dt.bfloat16
AF = mybir.ActivationFunctionType
MUL = mybir.AluOpType.mult
ADD = mybir.AluOpType.add
SUB = mybir.AluOpType.subtract

S, B, DM, H, DH = 2048, 2, 2048, 16, 128
NC = 8                 # cores
HC = H // NC           # heads per core = 2
JC = HC * DH           # per-core inner dim = 256
T = S * B              # tokens = 4096
KO = DM // 128         # contraction chunks = 16
TCH = T // 128         # token chunks = 32
SCH = S // 128         # per-batch chunks = 16
EPS = 1e-6
BF = ml_dtypes.bfloat16

_CACHE = {}


def _build(g_ones):
    nc = bacc.Bacc(trn_type="TRN2", target_bir_lowering=False, debug=False,
                   num_devices=NC)

    xT_d = nc.dram_tensor("xT", [TCH, 128, KO, 128], BF16,
                          kind="ExternalInput").ap()
    wqkv_d = nc.dram_tensor("wqkv", [DM, 3 * JC], BF16,
                            kind="ExternalInput").ap()
    wo_d = nc.dram_tensor("woT", [JC, DM], BF16, kind="ExternalInput").ap()
    rope_d = nc.dram_tensor("rope", [S, DH // 2], F32,
                            kind="ExternalInput").ap()
    gq_d = nc.dram_tensor("gq", [1, DH], F32, kind="ExternalInput").ap()
    gk_d = nc.dram_tensor("gk", [1, DH], F32, kind="ExternalInput").ap()
    out_d = nc.dram_tensor("out", [T, DM], BF16, kind="ExternalOutput").ap()

    with tile.TileContext(nc) as tc:
        with tc.tile_pool(name="persist", bufs=1) as persist:
            qT = persist.tile([128, HC, T], BF16)    # d on partitions
            kT = persist.tile([128, HC, T], BF16)
            v_sb = persist.tile([128, TCH, JC], BF16)  # tokens on partitions
            wo = persist.tile([128, HC, DM], BF16)

            # ---------------- Phase A: projections + norm + rope ----------
            with tc.tile_pool(name="pha", bufs=1) as pha, \
                 tc.tile_pool(name="wka", bufs=3) as wka, \
                 tc.tile_pool(name="xin", bufs=4) as xin, \
                 tc.tile_pool(name="ppqk", bufs=2, space="PSUM") as ppqk, \
                 tc.tile_pool(name="ppv", bufs=2, space="PSUM") as ppv, \
                 tc.tile_pool(name="pptr", bufs=2, space="PSUM") as pptr:

                # prefetch: first x chunk before the weights so the PE can
                # start as soon as wqkv[0] lands.
                def load_xc(tcch):
                    t = xin.tile([128, KO, 128], BF16, tag="xc")
                    eng = nc.scalar if tcch % 2 == 0 else nc.sync
                    eng.dma_start(t[:], xT_d[tcch])
                    return t

                # dummy Sin on a ready tile: pulls the ACT sin table set in
                # at t=0, concurrent with the rope DMA, so the real Sins
                # (head of the strict-FIFO ACT queue) don't pay the load.
                dmy = pha.tile([128, 1], F32)
                nc.vector.memset(dmy[:], 0.0)
                dmy2 = pha.tile([128, 1], F32)
                nc.scalar.activation(dmy2[:], dmy[:], AF.Sin, bias=0.0)

                # rope angles first (small, feeds the Sin -> rope chain that
                # gates the first chunks' SBUF recycling), on the queue the
                # first x chunk does NOT use.
                rope_sb = pha.tile([128, SCH, 64], F32)
                nc.sync.dma_start(
                    rope_sb[:], rope_d.rearrange("(rc p) d -> p rc d", p=128))

                xc_next = load_xc(0)
                xc_q = [load_xc(1), load_xc(2)]

                wqkv_src = wqkv_d.rearrange("(ko ki) n -> ki ko n", ki=128)
                wqkv = []
                for ko in range(KO):
                    wk_t = pha.tile([128, 3 * JC], BF16, tag=f"wqkv{ko}")
                    eng = nc.sync if ko % 2 == 0 else nc.scalar
                    eng.dma_start(wk_t[:], wqkv_src[:, ko, :])
                    wqkv.append(wk_t)

                if not g_ones:
                    g_sb = pha.tile([1, 2, DH], F32)
                    nc.scalar.dma_start(g_sb[:, 0, :], gq_d[:])
                    nc.scalar.dma_start(g_sb[:, 1, :], gk_d[:])

                epsb = pha.tile([128, 1], F32)
                nc.vector.memset(epsb[:], float(DH * EPS))
                ident = pha.tile([128, 128], BF16)
                make_identity(nc, ident[:])

                # ACT Sin needs args in [-pi, pi]. Single fold (valid for
                # |x + shift| < 3pi; angles are O(1) randn):
                PI, TWOPI = float(np.pi), float(2 * np.pi)

                def wrapped_sin(dst, shift):
                    xs = pha.tile([128, SCH, 64], F32, tag="w_xs")
                    if shift:
                        nc.vector.tensor_scalar_add(xs[:], rope_sb[:], shift)
                    else:
                        nc.vector.tensor_copy(xs[:], rope_sb[:])
                    hi = pha.tile([128, SCH, 64], F32, tag="w_m")
                    nc.vector.tensor_scalar(hi[:], xs[:], PI, TWOPI,
                                            mybir.AluOpType.is_gt, MUL)
                    nc.vector.tensor_tensor(xs[:], xs[:], hi[:], SUB)
                    lo = pha.tile([128, SCH, 64], F32, tag="w_m")
                    nc.vector.tensor_scalar(lo[:], xs[:], -PI, TWOPI,
                                            mybir.AluOpType.is_lt, MUL)
                    nc.vector.tensor_tensor(xs[:], xs[:], lo[:], ADD)
                    nc.scalar.activation(dst[:], xs[:], AF.Sin, bias=0.0)

                if g_ones:
                    # RMSNorm weights are 1: skip the factor fold entirely;
                    # rope reads the bf16 cos/sin tiles directly and the
                    # first chunk's rope only waits on the Sin chain.
                    cos_sb = pha.tile([128, SCH, 64], BF16)
                    sin_sb = pha.tile([128, SCH, 64], BF16)
                    wrapped_sin(sin_sb, 0.0)
                    wrapped_sin(cos_sb, float(np.pi / 2))
                else:
                    cos_f = pha.tile([128, SCH, 64], F32)
                    sin_f = pha.tile([128, SCH, 64], F32)
                    wrapped_sin(sin_f, 0.0)
                    wrapped_sin(cos_f, float(np.pi / 2))

                    # fold the RMSNorm weights into the rotation factors:
                    # o1 = x1*(g1*c) - x2*(g2*s); o2 = x1*(g1*s) + x2*(g2*c)
                    C1 = pha.tile([128, SCH, 2, 64], BF16)
                    S1 = pha.tile([128, SCH, 2, 64], BF16)
                    C2 = pha.tile([128, SCH, 2, 64], BF16)
                    S2 = pha.tile([128, SCH, 2, 64], BF16)
                    gb = pha.tile([128, 2, DH], F32)
                    nc.gpsimd.partition_broadcast(gb[:], g_sb[:])
                    for t in range(2):
                        g1 = gb[:, t, None, 0:64].broadcast_to(
                            (128, SCH, 64))
                        g2 = gb[:, t, None, 64:128].broadcast_to(
                            (128, SCH, 64))
                        nc.vector.tensor_tensor(C1[:, :, t, :], cos_f[:],
                                                g1, MUL)
                        nc.vector.tensor_tensor(S1[:, :, t, :], sin_f[:],
                                                g1, MUL)
                        nc.vector.tensor_tensor(C2[:, :, t, :], cos_f[:],
                                                g2, MUL)
                        nc.vector.tensor_tensor(S2[:, :, t, :], sin_f[:],
                                                g2, MUL)

                # transposes of chunk i are emitted after chunk i+2's
                # matmuls: the in-order PE queue then has matmul runway
                # while chunk 0/1's rope chain (cos/sin/cfac build) settles.
                pending_tr = []

                for tcch in range(TCH):
                    sc = tcch % SCH  # chunk index within batch (rope rows)
                    xc = xc_next
                    if xc_q:
                        xc_next = xc_q.pop(0)
                    if tcch + 3 < TCH:
                        xc_q.append(load_xc(tcch + 3))
                    if tcch == 8:
                        # wo is first needed by the output projection of
                        # (b0, sj0) — load it after the startup rush
                        wo_src = wo_d.rearrange("(h ki) n -> ki h n", ki=128)
                        for h in range(HC):
                            nc.sync.dma_start(wo[:, h, :], wo_src[:, h, :])

                    ps_qk = ppqk.tile([128, 2 * JC], F32, tag="psqk")
                    ps_v = ppv.tile([128, JC], F32, tag="psv")
                    for ko in range(KO):
                        nc.tensor.matmul(ps_qk[:], xc[:, ko, :],
                                         wqkv[ko][:, 0:2 * JC],
                                         start=(ko == 0), stop=(ko == KO - 1))
                        nc.tensor.matmul(ps_v[:], xc[:, ko, :],
                                         wqkv[ko][:, 2 * JC:3 * JC],
                                         start=(ko == 0), stop=(ko == KO - 1))
                    if len(pending_tr) > 2:
                        pending_tr.pop(0)()
                    nc.scalar.copy(v_sb[:, tcch, :], ps_v[:])

                    # rms stats over each head's 128 dims (q:2 heads, k:2)
                    ssq = wka.tile([128, 4], F32, tag="ssq")
                    sqd = wka.tile([128, DH], BF16, tag="sqd")
                    for th in range(4):
                        nc.scalar.activation(
                            sqd[:], ps_qk[:, th * DH:(th + 1) * DH],
                            AF.Square, accum_out=ssq[:, th:th + 1])
                    # q side folds 1/sqrt(DH): 1/sqrt(ssq + DH*eps)
                    rr = wka.tile([128, 4], F32, tag="rr")
                    nc.scalar.activation(rr[:], ssq[:], AF.Sqrt, bias=epsb[:])
                    rr2 = wka.tile([128, 4], F32, tag="rr2")
                    nc.vector.reciprocal(rr2[:], rr[:])
                    # k side: 1/sqrt(ssq/DH+eps) = sqrt(DH)/sqrt(ssq+DH*eps)
                    nc.vector.tensor_scalar_mul(rr2[:, 2:4], rr2[:, 2:4],
                                                float(math.sqrt(DH)))
                    rr2b = wka.tile([128, 4], BF16, tag="rr2b")
                    nc.vector.tensor_copy(rr2b[:], rr2[:])

                    qkc = wka.tile([128, 2, HC, 2, 64], BF16, tag="qkc")
                    nc.vector.tensor_copy(
                        qkc[:].rearrange("p t h f d -> p (t h f d)"),
                        ps_qk[:])

                    if g_ones:
                        c1 = (cos_sb[:, sc, None, None, :]
                              .broadcast_to((128, 2, HC, 64)))
                        s1 = (sin_sb[:, sc, None, None, :]
                              .broadcast_to((128, 2, HC, 64)))
                        c2, s2 = c1, s1
                    else:
                        c1 = (C1[:, sc, :, None, :]
                              .broadcast_to((128, 2, HC, 64)))
                        s1 = (S1[:, sc, :, None, :]
                              .broadcast_to((128, 2, HC, 64)))
                        c2 = (C2[:, sc, :, None, :]
                              .broadcast_to((128, 2, HC, 64)))
                        s2 = (S2[:, sc, :, None, :]
                              .broadcast_to((128, 2, HC, 64)))
                    x1 = qkc[:, :, :, 0, :]
                    x2 = qkc[:, :, :, 1, :]
                    t1 = wka.tile([128, 2, HC, 64], BF16, tag="t1")
                    t2 = wka.tile([128, 2, HC, 64], BF16, tag="t2")
                    tr = wka.tile([128, 2, HC, 2, 64], BF16, tag="tr")
                    nc.vector.tensor_tensor(t1[:], x1, c1, MUL)
                    nc.vector.tensor_tensor(t2[:], x2, s2, MUL)
                    nc.vector.tensor_tensor(tr[:, :, :, 0, :], t1[:], t2[:],
                                            SUB)
                    nc.vector.tensor_tensor(t1[:], x1, s1, MUL)
                    nc.vector.tensor_tensor(t2[:], x2, c2, MUL)
                    nc.vector.tensor_tensor(tr[:, :, :, 1, :], t1[:], t2[:],
                                            ADD)
                    trr = wka.tile([128, 2, HC, DH], BF16, tag="trr")
                    nc.vector.tensor_tensor(
                        trr[:].rearrange("p t h d -> p (t h) d"),
                        tr[:].rearrange("p t h f d -> p (t h) (f d)"),
                        rr2b[:, :, None].broadcast_to((128, 2 * HC, DH)),
                        MUL)
                    def make_tr(trr_t, tcch_):
                        def emit_tr():
                            for t in range(2):
                                dstT = qT if t == 0 else kT
                                for h in range(HC):
                                    ps_tr = pptr.tile([128, 128], BF16,
                                                      tag="pstr")
                                    nc.tensor.transpose(ps_tr[:],
                                                        trr_t[:, t, h, :],
                                                        ident[:])
                                    dsl = dstT[:, h,
                                               tcch_ * 128:(tcch_ + 1) * 128]
                                    if (t * HC + h) % 2 == 0:
                                        nc.scalar.copy(dsl, ps_tr[:])
                                    else:
                                        nc.vector.tensor_copy(dsl, ps_tr[:])
                        return emit_tr

                    pending_tr.append(make_tr(trr, tcch))
                while pending_tr:
                    pending_tr.pop(0)()
                # preload the ACT exp table set during the A->B drain so
                # phase B's first exp doesn't pay the ~1.3us table load
                nc.scalar.activation(dmy2[:], dmy[:], AF.Exp)

            # ---------------- Phase B/C: SDPA + output projection ---------
            with tc.tile_pool(name="phb", bufs=1) as phb, \
                 tc.tile_pool(name="etp", bufs=6) as etp, \
                 tc.tile_pool(name="esp", bufs=2) as esp, \
                 tc.tile_pool(name="otp", bufs=2) as otp, \
                 tc.tile_pool(name="wkb", bufs=2) as wkb, \
                 tc.tile_pool(name="osbp", bufs=6) as osbp, \
                 tc.tile_pool(name="ppsc", bufs=2, space="PSUM") as ppsc, \
                 tc.tile_pool(name="ppav", bufs=2, space="PSUM") as ppav, \
                 tc.tile_pool(name="ppx", bufs=2, space="PSUM") as ppx:

                ones128 = phb.tile([128, 128], BF16)
                nc.vector.memset(ones128[:], 1.0)

                # Output-projection work of block j is interleaved into
                # block j+1's score/AV stream as PE filler.
                pending_c = []
                copy_rr = [0]

                def emit_c(n):
                    for _ in range(min(n, len(pending_c))):
                        pending_c.pop(0)()

                def make_c_unit(outT_t, b, sj, mi, oj, osb_t):
                    def unit():
                        ps_o = ppx.tile([128, 512], F32, tag="x")
                        for h in range(HC):
                            nc.tensor.matmul(
                                ps_o[:],
                                outT_t[:, h, mi * 128:(mi + 1) * 128],
                                wo[:, h, oj * 512:(oj + 1) * 512],
                                start=(h == 0), stop=(h == HC - 1))
                        osl = osb_t[:, oj * 512:(oj + 1) * 512]
                        if copy_rr[0] % 4 == 0:
                            nc.scalar.copy(osl, ps_o[:])
                        else:
                            nc.vector.tensor_copy(osl, ps_o[:])
                        copy_rr[0] += 1
                        if oj == 3:
                            m0 = b * S + sj * 512 + mi * 128
                            deng = nc.sync if mi % 2 == 0 else nc.scalar
                            deng.dma_start(out_d[m0:m0 + 128, :], osb_t[:])
                    return unit

                for b in range(B):
                    for sj in range(4):  # 512-query blocks within batch b
                        s0 = b * S + sj * 512
                        outT = otp.tile([128, HC, 512], BF16, tag="outT")
                        # the two head-blocks run interleaved at pair
                        # granularity: each head's exp latency hides behind
                        # the other head's score/AV matmuls. ppav/esp hand
                        # out one buffer per head from the same 2-buf tag.
                        ps_av0 = ppav.tile([128, 512], F32, tag="psav")
                        ps_av1 = ppav.tile([128, 512], F32, tag="psav")
                        es0 = esp.tile([128, 2, 512], BF16, tag="es")
                        es1 = esp.tile([128, 2, 512], BF16, tag="es")
                        ps_av = [ps_av0, ps_av1]
                        es = [es0, es1]
                        eprev = [None, None]

                        def pe_av(h, eT_t, pp):
                            for i in range(2):
                                ti = 2 * pp + i
                                nc.tensor.matmul(
                                    ps_av[h][:],
                                    v_sb[:, b * SCH + ti,
                                         h * DH:(h + 1) * DH],
                                    eT_t[:, i, :],
                                    start=(ti == 0),
                                    stop=(ti == SCH - 1))

                        for p in range(8):
                            for h in range(HC):
                                ps_sc = ppsc.tile([128, 2, 512], F32,
                                                  tag="pssc")
                                for i in range(2):
                                    ti = 2 * p + i
                                    nc.tensor.matmul(
                                        ps_sc[:, i, :],
                                        kT[:, h, b * S + ti * 128:
                                           b * S + (ti + 1) * 128],
                                        qT[:, h, s0:s0 + 512],
                                        start=True, stop=True)
                                eT = etp.tile([128, 2, 512], BF16, tag="eT")
                                nc.scalar.activation(
                                    eT[:].rearrange("p a b -> p (a b)"),
                                    ps_sc[:].rearrange("p a b -> p (a b)"),
                                    AF.Exp)
                                emit_c(1)
                                # full-tile presum: es[h] accumulates the 8
                                # eT pairs as [128, 1024] adds (7 total)
                                if p == 1:
                                    nc.vector.tensor_tensor(
                                        es[h][:], eprev[h][:], eT[:], ADD)
                                elif p > 1:
                                    nc.vector.tensor_tensor(
                                        es[h][:], es[h][:], eT[:], ADD)
                                if eprev[h] is not None:
                                    pe_av(h, eprev[h], p - 1)
                                eprev[h] = eT
                        for h in range(HC):
                            pe_av(h, eprev[h], 7)

                        for h in range(HC):
                            # denominator, pre-broadcast across partitions:
                            # ps_db[i, q] = sum_k es[k, q] for every row i
                            # (allocated here, between C-unit allocations of
                            # the same pool tag, so the 2-buffer rotation
                            # never makes a C matmul wait on the recip)
                            ps_db = ppx.tile([128, 512], F32, tag="x")
                            nc.tensor.matmul(ps_db[:], ones128[:],
                                             es[h][:, 0, :],
                                             start=True, stop=False)
                            nc.tensor.matmul(ps_db[:], ones128[:],
                                             es[h][:, 1, :],
                                             start=False, stop=True)
                            recb = wkb.tile([128, 512], F32, tag="recb")
                            nc.vector.reciprocal_approx_fast(recb[:],
                                                             ps_db[:])
                            nc.vector.tensor_tensor(
                                outT[:, h, :], ps_av[h][:], recb[:], MUL)

                        for mi in range(4):
                            osb_t = osbp.tile([128, DM], BF16, tag="osb")
                            for oj in range(4):
                                pending_c.append(
                                    make_c_unit(outT, b, sj, mi, oj, osb_t))
                emit_c(len(pending_c))

    nc.compile()
    return nc


def _get_program(g_ones):
    key = ("prog", g_ones)
    if key not in _CACHE:
        _CACHE[key] = _build(g_ones)
    return _CACHE[key]


def _prep_inputs(x, rope_emb, Wq, Wk, Wv, Wo, gq, gk):
    x = np.asarray(x, dtype=np.float32)
    # b-major tokens: row r = b*S + s
    xbm = x.transpose(1, 0, 2).reshape(T, DM)
    xT = np.ascontiguousarray(
        xbm.reshape(TCH, 128, KO, 128).transpose(0, 3, 2, 1)).astype(BF)
    rope = np.ascontiguousarray(
        np.asarray(rope_emb, dtype=np.float32).reshape(S, DH)[:, :DH // 2])
    gq2 = np.asarray(gq, dtype=np.float32).reshape(1, DH)
    gk2 = np.asarray(gk, dtype=np.float32).reshape(1, DH)
    Wq = np.asarray(Wq, dtype=np.float32)
    Wk = np.asarray(Wk, dtype=np.float32)
    Wv = np.asarray(Wv, dtype=np.float32)
    Wo = np.asarray(Wo, dtype=np.float32)
    in_maps = []
    for c in range(NC):
        r0, r1 = c * JC, (c + 1) * JC
        wqkv = np.ascontiguousarray(np.concatenate(
            [Wq[r0:r1].T, Wk[r0:r1].T, Wv[r0:r1].T], axis=1)).astype(BF)
        woT = np.ascontiguousarray(Wo[:, r0:r1].T).astype(BF)
        in_maps.append({"xT": xT, "wqkv": wqkv, "woT": woT, "rope": rope,
                        "gq": gq2, "gk": gk2})
    g_ones = bool(np.all(gq2 == 1.0) and np.all(gk2 == 1.0))
    return in_maps, g_ones


def _gather(results):
    acc = results[0]["out"].astype(np.float32)
    for r in results[1:]:
        acc += r["out"].astype(np.float32)
    out = acc.reshape(B, S, DM).transpose(1, 0, 2)
    return np.ascontiguousarray(out)


def kernel(x, rope_emb, Wq, Wk, Wv, Wo, gq, gk):
    in_maps, g_ones = _prep_inputs(x, rope_emb, Wq, Wk, Wv, Wo, gq, gk)
    nc = _get_program(g_ones)
    res = bass_utils.run_bass_kernel_spmd(nc, in_maps,
                                          core_ids=list(range(NC)))
    return _gather(res.results)


def kernel_profiled(x, rope_emb, Wq, Wk, Wv, Wo, gq, gk):
    """Like kernel() but with NTFF tracing; returns (out, exec_time_ns)."""
    _install_ntff()
    in_maps, g_ones = _prep_inputs(x, rope_emb, Wq, Wk, Wv, Wo, gq, gk)
    nc = _get_program(g_ones)
    res = bass_utils.run_bass_kernel_spmd(nc, in_maps,
                                          core_ids=list(range(NC)),
                                          trace=True)
    return _gather(res.results), res.exec_time_ns


def _install_ntff():
    import contextlib
    import ctypes
    import types

    if "antenv.axon_hooks" in sys.modules:
        return
    so_path = "/opt/axon/libaxon_pjrt.so"
    try:
        lib = ctypes.CDLL(so_path)
    except OSError:
        return
    if not hasattr(lib, "axon_start_nrt_profile"):
        return
    lib.axon_start_nrt_profile.argtypes = [ctypes.POINTER(ctypes.c_int64),
                                           ctypes.c_size_t]
    lib.axon_start_nrt_profile.restype = ctypes.c_int64
    lib.axon_stop_nrt_profile.argtypes = [ctypes.c_char_p]
    lib.axon_stop_nrt_profile.restype = ctypes.c_int64

    @contextlib.contextmanager
    def hook(output_dir, device_ids):
        import jax
        jax.devices()
        if device_ids:
            ids = (ctypes.c_int64 * len(device_ids))(*device_ids)
            rc = lib.axon_start_nrt_profile(ids, len(device_ids))
        else:
            rc = lib.axon_start_nrt_profile(None, 0)
        if rc != 0:
            raise RuntimeError(f"axon_start_nrt_profile rc={rc}")
        try:
            yield
        finally:
            n = lib.axon_stop_nrt_profile(str(output_dir).encode())
            print(f"ntff profile: {n} file(s) -> {output_dir}", file=sys.stderr)

    mod = types.ModuleType("antenv.axon_hooks")
    _state = {"h": hook}
    mod.get_axon_ntff_profile_hook = lambda: _state["h"]
    mod.set_axon_ntff_profile_hook = lambda h: _state.__setitem__("h", h)
    sys.modules["antenv.axon_hooks"] = mod
